# revision 2
# baseline (speedup 1.0000x reference)
"""AdaptiveChebConv (K=3) distributed Bass kernel for 8 TRN2 NeuronCores.

Data-parallel over batch: B=16 -> 2 batches per core. adj/Theta replicated.

Per-core algorithm (per local batch b; N=1024, F=O=64, T=12):
  A  = adj * attn_b                      (DVE elementwise, fp32r)
  Z1 = A^T X                             (PE fp32r; X natural [n,(f,t)])
  Z2 = A^T Z1                            (PE fp32r)
  T_all = transpose_t(X|Z1|Z2)           (PE fp32r transposes -> [f,(t,n)] bf16)
  out[n,o,t] = relu(sum_k Theta_k^T-contraction)   (PE bf16, K=64 accum x3)
"""
import sys

if "/opt/trn_rl_repo" not in sys.path:
    sys.path.insert(0, "/opt/trn_rl_repo")

import numpy as np
from contextlib import ExitStack

import concourse.bass as bass
import concourse.tile as tile
from concourse import bacc, mybir
from concourse.bass_utils import run_bass_kernel_spmd

N_CORES = 8
B, N, F, T, K, O = 16, 1024, 64, 12, 3, 64
BL = B // N_CORES          # local batches per core = 2
NT = N // 128              # n-tiles = 8
FT = F * T                 # 768
OT = O * T                 # 768

F32 = mybir.dt.float32
F32R = mybir.dt.float32r
BF16 = mybir.dt.bfloat16

_NC = None


def _emit_batch(nc, pools, aps, b):
    """Emit one local batch's instructions."""
    (a_pool, big_pool, tall_pool, scr_pool, out_pool, zp, tp, qp) = pools
    (x_ap, attn_ap, adj_ap, out_ap, ident_t, theta_t) = aps

    # ---- A = adj * attn_b  (A_all [128, 8*1024] f32r; free = mt*1024 + n)
    A_all = a_pool.tile([128, NT * 1024], F32R, tag="A")
    nc.gpsimd.dma_start(
        A_all[:],
        attn_ap[b].rearrange("(mt p) n -> p mt n", p=128),
    )
    for mt in range(NT):
        adj_s = scr_pool.tile([128, 1024], F32, tag="adjscr")
        nc.sync.dma_start(adj_s[:], adj_ap[mt * 128:(mt + 1) * 128, :])
        sl = A_all[:, mt * 1024:(mt + 1) * 1024]
        nc.vector.tensor_mul(sl, sl.bitcast(F32), adj_s[:])

    # ---- X load (X_all [128, 8*768] f32r; free = nt*768 + f*12 + t)
    X_all = big_pool.tile([128, NT * FT], F32R, tag="X")
    nc.gpsimd.dma_start(
        X_all[:],
        x_ap[b].rearrange("(nt p) f t -> p nt (f t)", p=128),
    )

    # ---- T_all [64, 3*12288] bf16; free = j*12288 + t*1024 + n
    T_all = tall_pool.tile([64, 3 * T * N], BF16, tag="Tall")

    def emit_transposes(src_all, j):
        # src_all: [128, NT*FT] f32r, natural layout. 96 transposes in 24 packs.
        for nt in range(NT):
            src3 = src_all[:, nt * FT:(nt + 1) * FT].rearrange(
                "p (f t) -> p t f", t=T
            )  # [128, 12, 64]
            for g in range(3):  # groups of 4 t's
                pt = tp.tile([64, 512], F32R, tag="tp")
                for ti in range(4):
                    t = 4 * g + ti
                    nc.tensor.transpose(
                        pt[:, ti * 128:(ti + 1) * 128], src3[:, t, :], ident_t[:]
                    )
                dst = T_all[:, :].rearrange(
                    "p (j t n) -> p j t n", j=3, t=T
                )[:, j, 4 * g:4 * g + 4, nt * 128:(nt + 1) * 128]  # [64, 4, 128]
                src = pt[:].rearrange("p (t n) -> p t n", n=128)
                eng = nc.vector if (nt + g) % 2 == 0 else nc.scalar
                if eng is nc.vector:
                    nc.vector.tensor_copy(dst, src.bitcast(F32))
                else:
                    nc.scalar.activation(
                        dst, src.bitcast(F32), mybir.ActivationFunctionType.Copy
                    )

    def emit_big_matmul(dst_all, rhs_all):
        # dst = A^T rhs ; both [128, NT*FT] f32r natural layout
        for nt in range(NT):
            for ch in range(2):
                pz = zp.tile([128, 384], F32, tag="zp")
                for mt in range(NT):
                    nc.tensor.matmul(
                        pz[:],
                        A_all[:, mt * 1024 + nt * 128: mt * 1024 + (nt + 1) * 128],
                        rhs_all[:, mt * FT + ch * 384: mt * FT + (ch + 1) * 384],
                        start=(mt == 0),
                        stop=(mt == NT - 1),
                    )
                nc.vector.tensor_copy(
                    dst_all[:, nt * FT + ch * 384: nt * FT + (ch + 1) * 384], pz[:]
                )

    emit_transposes(X_all, 0)

    Z1_all = big_pool.tile([128, NT * FT], F32R, tag="Z1")
    emit_big_matmul(Z1_all, X_all)
    emit_transposes(Z1_all, 1)

    Z2_all = big_pool.tile([128, NT * FT], F32R, tag="Z2")
    emit_big_matmul(Z2_all, Z1_all)
    emit_transposes(Z2_all, 2)

    # ---- Theta pass: out[n, (o,t)] = relu(sum_j Theta_j^T-contract T_all_j)
    T4 = T_all[:, :].rearrange("p (j t n) -> p j t n", j=3, t=T)
    for nt in range(NT):
        o_tile = out_pool.tile([128, OT], F32, tag="out")
        for grp, (t0, tn) in enumerate(((0, 8), (8, 4))):
            pq = qp.tile([128, 512], F32, tag="qp")
            for ts in range(tn):
                t = t0 + ts
                for j in range(3):
                    nc.tensor.matmul(
                        pq[:, ts * 64:(ts + 1) * 64],
                        T4[:, j, t, nt * 128:(nt + 1) * 128],
                        theta_t[:, j * 64:(j + 1) * 64],
                        start=(j == 0),
                        stop=(j == 2),
                    )
            # relu-copy strided into natural (o,t) layout
            dst = o_tile[:].rearrange("p (o t) -> p t o", t=T)[:, t0:t0 + tn, :]
            src = pq[:, 0:tn * 64].rearrange("p (t o) -> p t o", o=64)
            nc.scalar.activation(dst, src, mybir.ActivationFunctionType.Relu)
        nc.sync.dma_start(
            out_ap[b, nt * 128:(nt + 1) * 128, :, :].rearrange("p o t -> p (o t)"),
            o_tile[:],
        )


def _build():
    nc = bacc.Bacc("TRN2", target_bir_lowering=False, debug=False)
    x_ap = nc.dram_tensor("x", [BL, N, F, T], F32, kind="ExternalInput").ap()
    attn_ap = nc.dram_tensor(
        "spatial_attention", [BL, N, N], F32, kind="ExternalInput"
    ).ap()
    adj_ap = nc.dram_tensor("adj", [N, N], F32, kind="ExternalInput").ap()
    theta_ap = nc.dram_tensor("Theta", [K, F, O], F32, kind="ExternalInput").ap()
    ident_ap = nc.dram_tensor("ident", [128, 128], F32, kind="ExternalInput").ap()
    out_ap = nc.dram_tensor("out", [BL, N, O, T], F32, kind="ExternalOutput").ap()

    with tile.TileContext(nc) as tc, ExitStack() as ctx:
        a_pool = ctx.enter_context(tc.tile_pool(name="apool", bufs=1))
        big_pool = ctx.enter_context(tc.tile_pool(name="big", bufs=1))
        tall_pool = ctx.enter_context(tc.tile_pool(name="tall", bufs=1))
        scr_pool = ctx.enter_context(tc.tile_pool(name="scr", bufs=2))
        out_pool = ctx.enter_context(tc.tile_pool(name="outp", bufs=3))
        const_pool = ctx.enter_context(tc.tile_pool(name="const", bufs=1))
        zp = ctx.enter_context(tc.tile_pool(name="zp", bufs=3, space="PSUM"))
        tp = ctx.enter_context(tc.tile_pool(name="tp", bufs=2, space="PSUM"))
        qp = ctx.enter_context(tc.tile_pool(name="qp", bufs=2, space="PSUM"))

        ident_t = const_pool.tile([128, 128], F32R, tag="ident")
        nc.gpsimd.dma_start(ident_t[:], ident_ap[:])
        theta_t = const_pool.tile([64, K * O], BF16, tag="theta")
        nc.gpsimd.dma_start(theta_t[:], theta_ap.rearrange("k f o -> f k o"))

        pools = (a_pool, big_pool, tall_pool, scr_pool, out_pool, zp, tp, qp)
        aps = (x_ap, attn_ap, adj_ap, out_ap, ident_t, theta_t)
        for b in range(BL):
            _emit_batch(nc, pools, aps, b)

    nc.compile()
    return nc


def kernel(**inputs):
    global _NC
    if _NC is None:
        _NC = _build()
    nc = _NC

    x = np.ascontiguousarray(inputs["x"], dtype=np.float32)
    attn = np.ascontiguousarray(inputs["spatial_attention"], dtype=np.float32)
    adj = np.ascontiguousarray(inputs["adj"], dtype=np.float32)
    theta = np.ascontiguousarray(inputs["Theta"], dtype=np.float32)
    ident = np.eye(128, dtype=np.float32)

    in_maps = []
    for i in range(N_CORES):
        s = slice(i * BL, (i + 1) * BL)
        in_maps.append(
            {
                "x": x[s],
                "spatial_attention": attn[s],
                "adj": adj,
                "Theta": theta,
                "ident": ident,
            }
        )
    res = run_bass_kernel_spmd(nc, in_maps, core_ids=list(range(N_CORES)))
    out = np.concatenate([res.results[i]["out"] for i in range(N_CORES)], axis=0)
    return out


# revision 4
# speedup vs baseline: 1.2324x; 1.2324x over previous
"""AdaptiveChebConv (K=3) distributed Bass kernel for 8 TRN2 NeuronCores.

Data-parallel over batch: B=16 -> 2 batches per core. adj/Theta replicated.

Per-core algorithm (per local batch b; N=1024, F=O=64, T=12):
  A  = adj * attn_b                      (DVE elementwise, bf16 out)
  Z1 = A^T X                             (PE bf16; X natural [n,(f,t)])
  Z2 = A^T Z1                            (PE bf16)
  T[j] = transpose_t(X|Z1|Z2)            (PE bf16 transposes -> [f,(t,n)])
  out[n,o,t] = relu(sum_j Theta_j^T-contraction)   (PE bf16, K=64 accum x3)
"""
import sys

if "/opt/trn_rl_repo" not in sys.path:
    sys.path.insert(0, "/opt/trn_rl_repo")

import numpy as np
from contextlib import ExitStack

import concourse.bass as bass
import concourse.tile as tile
from concourse import bacc, mybir
from concourse.bass_utils import run_bass_kernel_spmd

N_CORES = 8
B, N, F, T, K, O = 16, 1024, 64, 12, 3, 64
BL = B // N_CORES          # local batches per core = 2
NT = N // 128              # n-tiles = 8
FT = F * T                 # 768
OT = O * T                 # 768
XSPLIT = 4                 # x DMA split (n-tile pairs) for startup pipelining

F32 = mybir.dt.float32
BF16 = mybir.dt.bfloat16

_NC = None


def _emit_batch(nc, pools, aps, b):
    (a_pool, big_pool, t_pool, scr_pool, out_pool, zp, tp, qp) = pools
    (x_ap, attn_ap, adj_ap, out_ap, ident_t, theta_t) = aps

    # ---- X load: 4 tiles of 2 n-tiles each, bf16 cast DMA
    X_t = []
    for xs in range(XSPLIT):
        xt = big_pool.tile([128, 2 * FT], BF16, tag=f"X{xs}")
        nc.gpsimd.dma_start(
            xt[:],
            x_ap[b, xs * 256:(xs + 1) * 256].rearrange(
                "(nt p) f t -> p nt (f t)", p=128
            ),
        )
        X_t.append(xt)

    def x_slice(nt, lo, size):
        return X_t[nt // 2][:, (nt % 2) * FT + lo: (nt % 2) * FT + lo + size]

    # ---- A = adj * attn_b  (A_all [128, 8*1024] bf16; free = mt*1024 + n)
    A_all = a_pool.tile([128, NT * 1024], BF16, tag="A")
    for mt in range(NT):
        attn_s = scr_pool.tile([128, 1024], F32, tag="attnscr")
        nc.sync.dma_start(attn_s[:], attn_ap[b, mt * 128:(mt + 1) * 128, :])
        adj_s = scr_pool.tile([128, 1024], F32, tag="adjscr")
        nc.sync.dma_start(adj_s[:], adj_ap[mt * 128:(mt + 1) * 128, :])
        nc.vector.tensor_mul(
            A_all[:, mt * 1024:(mt + 1) * 1024], attn_s[:], adj_s[:]
        )

    # ---- T tiles: per (j, nt) [64, T*128] bf16; free = t*128 + n_local
    T_t = {}
    for j in range(3):
        for nt in range(NT):
            T_t[(j, nt)] = t_pool.tile(
                [64, T * 128], BF16, tag=f"T{j}_{nt}", name=f"T{j}_{nt}"
            )

    def emit_transposes(src_fn, j):
        # src_fn(nt) -> [128, FT] bf16 natural slice; 96 transposes in 24 packs
        for nt in range(NT):
            src3 = src_fn(nt).rearrange("p (f t) -> p t f", t=T)  # [128,12,64]
            for g in range(3):
                pt = tp.tile([64, 512], BF16, tag="tp")
                for ti in range(4):
                    t = 4 * g + ti
                    nc.tensor.transpose(
                        pt[:, ti * 128:(ti + 1) * 128], src3[:, t, :], ident_t[:]
                    )
                eng = nc.vector if (nt + g) % 2 == 0 else nc.scalar
                dst = T_t[(j, nt)][:, g * 512:(g + 1) * 512]
                if eng is nc.vector:
                    nc.vector.tensor_copy(dst, pt[:])
                else:
                    nc.scalar.activation(
                        dst, pt[:], mybir.ActivationFunctionType.Copy
                    )

    def emit_big_matmul(dst_all, rhs_fn):
        # dst = A^T rhs ; natural layout [128, NT*FT]
        for nt in range(NT):
            for ch in range(2):
                pz = zp.tile([128, 384], F32, tag="zp")
                for mt in range(NT):
                    nc.tensor.matmul(
                        pz[:],
                        A_all[:, mt * 1024 + nt * 128: mt * 1024 + (nt + 1) * 128],
                        rhs_fn(mt, ch * 384, 384),
                        start=(mt == 0),
                        stop=(mt == NT - 1),
                    )
                nc.vector.tensor_copy(
                    dst_all[:, nt * FT + ch * 384: nt * FT + (ch + 1) * 384], pz[:]
                )

    emit_transposes(lambda nt: x_slice(nt, 0, FT), 0)

    Z1_all = big_pool.tile([128, NT * FT], BF16, tag="Z1")
    emit_big_matmul(Z1_all, x_slice)
    emit_transposes(lambda nt: Z1_all[:, nt * FT:(nt + 1) * FT], 1)

    Z2_all = big_pool.tile([128, NT * FT], BF16, tag="Z2")
    emit_big_matmul(Z2_all, lambda mt, lo, sz: Z1_all[:, mt * FT + lo: mt * FT + lo + sz])
    emit_transposes(lambda nt: Z2_all[:, nt * FT:(nt + 1) * FT], 2)

    # ---- Theta pass: out[n, (o,t)] = relu(sum_j Theta_j^T-contract T_j)
    for nt in range(NT):
        o_tile = out_pool.tile([128, OT], F32, tag="out")
        for (t0, tn) in ((0, 8), (8, 4)):
            pq = qp.tile([128, 512], F32, tag="qp")
            for ts in range(tn):
                t = t0 + ts
                for j in range(3):
                    nc.tensor.matmul(
                        pq[:, ts * 64:(ts + 1) * 64],
                        T_t[(j, nt)][:, t * 128:(t + 1) * 128],
                        theta_t[:, j * 64:(j + 1) * 64],
                        start=(j == 0),
                        stop=(j == 2),
                    )
            dst = o_tile[:].rearrange("p (o t) -> p t o", t=T)[:, t0:t0 + tn, :]
            src = pq[:, 0:tn * 64].rearrange("p (t o) -> p t o", o=64)
            nc.scalar.activation(dst, src, mybir.ActivationFunctionType.Relu)
        nc.sync.dma_start(
            out_ap[b, nt * 128:(nt + 1) * 128, :, :].rearrange("p o t -> p (o t)"),
            o_tile[:],
        )


def _build():
    nc = bacc.Bacc("TRN2", target_bir_lowering=False, debug=False)
    x_ap = nc.dram_tensor("x", [BL, N, F, T], F32, kind="ExternalInput").ap()
    attn_ap = nc.dram_tensor(
        "spatial_attention", [BL, N, N], F32, kind="ExternalInput"
    ).ap()
    adj_ap = nc.dram_tensor("adj", [N, N], F32, kind="ExternalInput").ap()
    theta_ap = nc.dram_tensor("Theta", [K, F, O], F32, kind="ExternalInput").ap()
    ident_ap = nc.dram_tensor("ident", [128, 128], F32, kind="ExternalInput").ap()
    out_ap = nc.dram_tensor("out", [BL, N, O, T], F32, kind="ExternalOutput").ap()

    with tile.TileContext(nc) as tc, ExitStack() as ctx:
        a_pool = ctx.enter_context(tc.tile_pool(name="apool", bufs=2))
        big_pool = ctx.enter_context(tc.tile_pool(name="big", bufs=2))
        t_pool = ctx.enter_context(tc.tile_pool(name="tpool", bufs=1))
        scr_pool = ctx.enter_context(tc.tile_pool(name="scr", bufs=2))
        out_pool = ctx.enter_context(tc.tile_pool(name="outp", bufs=3))
        const_pool = ctx.enter_context(tc.tile_pool(name="const", bufs=1))
        zp = ctx.enter_context(tc.tile_pool(name="zp", bufs=3, space="PSUM"))
        tp = ctx.enter_context(tc.tile_pool(name="tp", bufs=3, space="PSUM"))
        qp = ctx.enter_context(tc.tile_pool(name="qp", bufs=2, space="PSUM"))

        ident_t = const_pool.tile([128, 128], BF16, tag="ident")
        nc.gpsimd.dma_start(ident_t[:], ident_ap[:])
        theta_t = const_pool.tile([64, K * O], BF16, tag="theta")
        nc.gpsimd.dma_start(theta_t[:], theta_ap.rearrange("k f o -> f k o"))

        pools = (a_pool, big_pool, t_pool, scr_pool, out_pool, zp, tp, qp)
        aps = (x_ap, attn_ap, adj_ap, out_ap, ident_t, theta_t)
        for b in range(BL):
            _emit_batch(nc, pools, aps, b)

    nc.compile()
    return nc


def kernel(**inputs):
    global _NC
    if _NC is None:
        _NC = _build()
    nc = _NC

    x = np.ascontiguousarray(inputs["x"], dtype=np.float32)
    attn = np.ascontiguousarray(inputs["spatial_attention"], dtype=np.float32)
    adj = np.ascontiguousarray(inputs["adj"], dtype=np.float32)
    theta = np.ascontiguousarray(inputs["Theta"], dtype=np.float32)
    ident = np.eye(128, dtype=np.float32)

    in_maps = []
    for i in range(N_CORES):
        s = slice(i * BL, (i + 1) * BL)
        in_maps.append(
            {
                "x": x[s],
                "spatial_attention": attn[s],
                "adj": adj,
                "Theta": theta,
                "ident": ident,
            }
        )
    res = run_bass_kernel_spmd(nc, in_maps, core_ids=list(range(N_CORES)))
    out = np.concatenate([res.results[i]["out"] for i in range(N_CORES)], axis=0)
    return out
